# revision 8
# baseline (speedup 1.0000x reference)
"""Trainium2 Bass kernel for nn_AtomicPositionalEncoding (v2).

kernel(**inputs): FULL x [256,1024,4] f32 -> FULL out [256,1024,128] f32.
Shards batch across 8 NeuronCores (32 examples each), one SPMD Bass program.

v2 design vs baseline:
- Point mapping n = 8*p + t (partition-major): input loads via direct
  strided DMA (no PE shuffle); output DMA per example is one 2KB
  contiguous chunk per partition -> full DMA efficiency at bf16.
- Output in bf16 (halves HBM write traffic); host casts back to f32.
- Epilogue per example: X = mask*pgi via 4 per-k tensor_tensor ops with
  all operands bf16 + packed pair views (DVE 2x_1p mode), then
  out = X + negmistd via one pair-packed 2x add (or per-tile act ops /
  gpsimd ops per the engine split tables).
- G gathered directly in point layout: G_A[pt,(q,k)] =
  matmul(weights=onehotT_tile, rhs=squad) -- no layout-B roundtrip.
- Stats computed per pair of quads; var folded as
  std = sqrt(max(msq - mean^2/D, 0)/(D-1)); fast approx reciprocals.

Column order: j = (b, t), b major: quad jq owns cols 32*jq..32*jq+31,
ordered (q, t) within the quad. masks/prodm2 stored t-major per quad
so hist/transpose slices are contiguous.
"""

import os
import sys

import numpy as np

for p in ("/opt/trn_rl_repo", "/root/.axon_site/_ro/trn_rl_repo"):
    if os.path.isdir(p) and p not in sys.path:
        sys.path.insert(0, p)

import concourse.bass as bass
import concourse.bacc as bacc
import concourse.mybir as mybir
from concourse.tile import TileContext

F32 = mybir.dt.float32
BF16 = mybir.dt.bfloat16

EX = 32          # examples per core
NPT = 1024       # points per example
T_PER_EX = 8     # 8 tiles of 128 points per example (t = n % 8)
NCOL = EX * T_PER_EX          # 256 point-tile columns
NQ = 8                        # quads of 4 examples
C = 32
K = 4
D = 128
ETA = 4.0
RC = 6.0
Y00 = 0.5 / np.sqrt(np.pi)
C1 = np.sqrt(3.0 / (4.0 * np.pi))

AF = mybir.ActivationFunctionType
OP = mybir.AluOpType

# ---- tunable engine splits ----
# X = mask*pgi expansion engine per example ('ve' = 4 per-k 2x ops, 'gp' = 1 op)
X_ENG = (['ve', 've', 've', 'gp'] * 8)
# out = X + negmistd engine per example ('ve' = 1 pair-packed 2x op,
# 'act' = 8 per-tile Identity+bias ops)
ADD_ENG = (['ve', 've', 'act', 've'] * 4 + ['ve', 'act', 'act', 've'] * 2
           + ['ve', 've', 'act', 've'] * 2)


def _consts_f32() -> np.ndarray:
    iota32 = np.tile(np.arange(C, dtype=np.float32), (128, 1))          # [128,32]
    blockmask = np.zeros((128, 16), dtype=np.float32)                   # [128,16]
    for pp_ in range(128):
        for f in range(16):
            if pp_ // 32 == f // 4:
                blockmask[pp_, f] = 1.0
    ident = np.eye(128, dtype=np.float32)                               # [128,128]
    bconst = np.tile(np.array([np.pi / 2, -1.5, -3.0, -4.5], np.float32), (128, 1))
    return np.concatenate(
        [iota32.ravel(), blockmask.ravel(), ident.ravel(), bconst.ravel()]
    )


CF_SIZES = [128 * 32, 128 * 16, 128 * 128, 128 * 4]
CF_TOTAL = sum(CF_SIZES)


def build_nc() -> bass.Bass:
    nc = bacc.Bacc()
    # x pre-permuted on host: xp[p, b, t, c] = x[b, 8p+t, c]
    x_d = nc.dram_tensor("x", [128, EX * T_PER_EX * 4], F32, kind="ExternalInput")
    cf_d = nc.dram_tensor("cf", [CF_TOTAL], F32, kind="ExternalInput")
    out_d = nc.dram_tensor("out", [EX, NPT, D], BF16, kind="ExternalOutput")

    with TileContext(nc) as tc:
        with (
            tc.tile_pool(name="persist", bufs=1) as pp,
            tc.tile_pool(name="ohp", bufs=3) as ohp,
            tc.tile_pool(name="outp", bufs=6) as op_,
            tc.tile_pool(name="xep", bufs=6) as xep,
            tc.tile_pool(name="ph", bufs=2, space="PSUM") as ph,       # hist
            tc.tile_pool(name="poh", bufs=2, space="PSUM") as poh,     # onehotT
            tc.tile_pool(name="pga", bufs=2, space="PSUM") as pga,     # G_A
        ):
            ve, act, gp, pe, sy = nc.vector, nc.scalar, nc.gpsimd, nc.tensor, nc.sync

            # ---- constants ----
            offs = np.cumsum([0] + CF_SIZES)
            def cslice(i, shape):
                t = pp.tile(shape, F32, name=f"const{i}", tag=f"const{i}")
                src = cf_d[offs[i]:offs[i + 1]].rearrange("(p f) -> p f", p=shape[0])
                sy.dma_start(t, src)
                return t
            iota32 = cslice(0, [128, 32])
            blockmask = cslice(1, [128, 16])
            ident = cslice(2, [128, 128])
            bconst = cslice(3, [128, 4])
            ident16 = pp.tile([128, 128], BF16, name="ident16", tag="ident16")
            ve.tensor_copy(ident16, ident)

            # ---- load x (host pre-permuted to (p, (b t c))) ----
            x_sb = pp.tile([128, NCOL * 4], F32, name="x", tag="x")
            sy.dma_start(x_sb, x_d[:, :])
            x4 = x_sb.rearrange("p (j c) -> p j c", c=4)
            xyz = x4[:, :, 0:3]
            clsf2 = x4[:, :, 3:4].rearrange("p j one -> p (j one)")   # [128,256]

            # ---- stage 1 (per point) ----
            sq = pp.tile([128, NCOL * 3], F32, name="sq", tag="sq").rearrange("p (j c) -> p j c", c=3)
            act.activation(sq, xyz, AF.Square)
            r2 = pp.tile([128, NCOL], F32, name="r2", tag="r2")
            ve.tensor_reduce(r2, sq, mybir.AxisListType.X, OP.add)
            r = pp.tile([128, NCOL], F32, name="r", tag="r")
            act.activation(r, r2, AF.Sqrt)
            rinv = pp.tile([128, NCOL], F32, name="rinv", tag="rinv")
            ve.reciprocal_approx_fast(rinv, r)
            rmin = pp.tile([128, NCOL], F32, name="rmin", tag="rmin")
            ve.tensor_scalar(out=rmin, in0=r, scalar1=float(RC), scalar2=None, op0=OP.min)
            cosv = pp.tile([128, NCOL], F32, name="cosv", tag="cosv")
            act.activation(cosv, rmin, AF.Sin, bias=bconst[:, 0:1], scale=float(-np.pi / RC))
            env = pp.tile([128, NCOL], F32, name="env", tag="env")
            ve.tensor_scalar(out=env, in0=cosv, scalar1=0.5, scalar2=0.5, op0=OP.mult, op1=OP.add)

            # group Squares then Exps to avoid act-table thrash; Exp in place
            radial = pp.tile([128, NCOL * K], F32, name="radial", tag="radial").rearrange("p (j k) -> p j k", k=K)
            for k in range(K):
                bias_k = 0.0 if k == 0 else bconst[:, k:k + 1]
                act.activation(radial[:, :, k:k + 1].rearrange("p j one -> p (j one)"),
                               r, AF.Square, bias=bias_k, scale=1.0)
            for k in range(K):
                rk = radial[:, :, k:k + 1].rearrange("p j one -> p (j one)")
                act.activation(rk, rk, AF.Exp, scale=float(-ETA))
            m = pp.tile([128, NCOL * K], F32, name="m", tag="m").rearrange("p (j k) -> p j k", k=K)
            env_b = env.unsqueeze(2).broadcast_to([128, NCOL, K])
            ve.tensor_tensor(m, radial, env_b, OP.mult)

            u3 = pp.tile([128, NCOL * 3], F32, name="u3", tag="u3").rearrange("p (j c) -> p j c", c=3)
            rinv_b = rinv.unsqueeze(2).broadcast_to([128, NCOL, 3])
            ve.tensor_tensor(u3, xyz, rinv_b, OP.mult)
            prodm = pp.tile([128, NCOL * K], F32, name="prodm", tag="prodm").rearrange("p (j k) -> p j k", k=K)
            ve.tensor_scalar(out=prodm[:, :, 0:1], in0=m[:, :, 0:1], scalar1=float(Y00),
                             scalar2=None, op0=OP.mult)
            ve.scalar_tensor_tensor(out=prodm[:, :, 1:3], in0=m[:, :, 1:3], scalar=float(C1),
                                    in1=u3[:, :, 1:3], op0=OP.mult, op1=OP.mult)
            ve.scalar_tensor_tensor(out=prodm[:, :, 3:4], in0=m[:, :, 3:4], scalar=float(C1),
                                    in1=u3[:, :, 0:1], op0=OP.mult, op1=OP.mult)
            # prodm^2 in bf16, t-major per quad: [128, (jq, t, q, k)]
            prodm2q = pp.tile([128, NCOL * K], BF16, name="prodm2", tag="prodm2") \
                        .rearrange("p (jq t q k) -> p jq t q k", jq=NQ, t=T_PER_EX, q=4)

            # ---- masks: onehot(cls) bf16, storage [128, (jq, t, q, c)] ----
            masks_flat = pp.tile([128, NCOL * C], BF16, name="masks", tag="masks")
            masks5 = masks_flat.rearrange("p (jq t q c) -> p jq t q c",
                                          jq=NQ, t=T_PER_EX, q=4, c=C)
            # bf16 iota + pair-expanded cls so is_equal runs in 2x mode
            iota16 = pp.tile([128, C], BF16, name="iota16", tag="iota16")
            ve.tensor_copy(iota16, iota32)
            iota16w = iota16.rearrange("p (c2 w) -> p c2 w", w=2)
            clsX = pp.tile([128, NCOL * 2], BF16, name="clsX", tag="clsX")
            clsX3 = clsX.rearrange("p (j two) -> p j two", two=2)
            ve.tensor_copy(clsX3, clsf2.unsqueeze(2).broadcast_to([128, NCOL, 2]))

            # ---- persistent tensors for the quad pipeline ----
            squadh = pp.tile([128, NQ * 16], BF16, name="squadh", tag="squadh")
            g_all = pp.tile([128, NQ * 128], F32, name="g_all", tag="g_all")
            pg = pp.tile([128, NCOL * K], F32, name="pg", tag="pg")
            pg3 = pg.rearrange("p (j k) -> p j k", k=K)
            pg2 = pp.tile([128, NCOL * K], F32, name="pg2", tag="pg2").rearrange("p (j k) -> p j k", k=K)
            pgi3 = pp.tile([128, NCOL * K], F32, name="pgi", tag="pgi").rearrange("p (j k) -> p j k", k=K)
            # pair-expanded pgi (bf16) for the 2x per-k expansion:
            # pgiX[p, j, k, w] = pgi[p, j, k], w in {0,1}
            pgiX = pp.tile([128, NCOL * K * 2], BF16, name="pgiX", tag="pgiX")
            pgiX4 = pgiX.rearrange("p (j k w) -> p j k w", k=K, w=2)
            mean = pp.tile([128, NCOL], F32, name="mean", tag="mean")
            msq = pp.tile([128, NCOL], F32, name="msq", tag="msq")
            m2 = pp.tile([128, NCOL], F32, name="m2", tag="m2")
            var = pp.tile([128, NCOL], F32, name="var", tag="var")
            std = pp.tile([128, NCOL], F32, name="std", tag="std")
            istd = pp.tile([128, NCOL], F32, name="istd", tag="istd")
            negmistd = pp.tile([128, NCOL], F32, name="negmistd", tag="negmistd")
            # pair-expanded negmistd for the 4x fill: [128, (j, 2)] bf16
            nmX = pp.tile([128, NCOL * 2], BF16, name="nmX", tag="nmX")
            nmX3 = nmX.rearrange("p (j two) -> p j two", two=2)

            # ---- per-quad pipeline with skew ----
            def preamble(jq):
                cs = slice(32 * jq, 32 * (jq + 1))        # this quad's 32 cols
                for q in range(4):
                    b = 4 * jq + q
                    # masks in 2x mode: all operands bf16, innermost packed pair
                    cls_w = clsX3[:, 8 * b:8 * b + 8, :].unsqueeze(2) \
                        .broadcast_to([128, T_PER_EX, C // 2, 2])
                    iota_w = iota16w.unsqueeze(1) \
                        .broadcast_to([128, T_PER_EX, C // 2, 2])
                    mq = masks5[:, jq, :, q, :] \
                        .rearrange("p t (c2 w) -> p t c2 w", w=2)
                    ve.tensor_tensor(mq, cls_w, iota_w, OP.is_equal)
                # prodm^2 for the whole quad in one op (strided view)
                pq = prodm[:, 32 * jq:32 * (jq + 1), :] \
                    .rearrange("p (q t) k -> p t q k", q=4)
                ve.tensor_tensor(prodm2q[:, jq], pq, pq, OP.mult)
                # hist[(q,c),(q',k)]: diag blocks = per-example hists
                hist_ps = ph.tile([128, 16], F32, name="hist_ps", tag="hist_ps")
                for t in range(T_PER_EX):
                    pe.matmul(hist_ps,
                              masks5[:, jq, t].rearrange("p a b -> p (a b)"),
                              prodm2q[:, jq, t].rearrange("p a b -> p (a b)"),
                              start=(t == 0), stop=(t == T_PER_EX - 1))
                # scale = 1/max(sqrt(hist),1e-12); off-diag junk masked by blockmask
                sr4 = pp.tile([128, 16], F32, name=f"sr4_{jq}", tag=f"sr4_{jq}")
                act.activation(sr4, hist_ps, AF.Sqrt)
                ve.tensor_scalar(out=sr4, in0=sr4, scalar1=1e-12, scalar2=None, op0=OP.max)
                ve.reciprocal_approx_fast(sr4, sr4)
                ve.tensor_tensor(squadh[:, 16 * jq:16 * (jq + 1)], sr4, blockmask, OP.mult)

                # onehotT tiles (PE transpose -> PSUM -> SBUF copy)
                oh_ps = poh.tile([128, NPT], BF16, name="oh_ps", tag="oh_ps")
                for t in range(T_PER_EX):
                    pe.transpose(oh_ps[:, 128 * t:128 * (t + 1)],
                                 masks5[:, jq, t].rearrange("p a b -> p (a b)"),
                                 ident16)
                oh_sb = ohp.tile([128, NPT], BF16, name="oh_sb", tag="oh_sb")
                act.copy(oh_sb, oh_ps)
                # G_A gather: per t, [128 pt, 16 (q,k)] = ohT_t @ squad
                ga_ps = pga.tile([128, 128], F32, name="ga_ps", tag="ga_ps")
                for t in range(T_PER_EX):
                    pe.matmul(ga_ps[:, 16 * t:16 * (t + 1)],
                              oh_sb[:, 128 * t:128 * (t + 1)],
                              squadh[:, 16 * jq:16 * (jq + 1)],
                              start=True, stop=True)
                # reorder (t,q,k) -> (q,t,k) into g_all
                ga4 = ga_ps.rearrange("p (t q k) -> p q t k", t=T_PER_EX, q=4)
                g4 = g_all[:, 128 * jq:128 * (jq + 1)] \
                    .rearrange("p (q t k) -> p q t k", q=4, t=T_PER_EX)
                ve.tensor_copy(g4, ga4)
            def stats_pair(jp):
                """Per-point normalization scalars for quads 2jp, 2jp+1
                (64 cols at once to halve DVE op count)."""
                cs = slice(64 * jp, 64 * (jp + 1))
                n = 64
                gq3 = g_all[:, 256 * jp:256 * (jp + 1)] \
                    .rearrange("p (j k) -> p j k", k=K)
                ve.tensor_tensor(pg3[:, cs, :], prodm[:, cs, :], gq3, OP.mult)
                ve.tensor_reduce(mean[:, cs], pg3[:, cs, :], mybir.AxisListType.X, OP.add)
                gp.tensor_tensor(pg2[:, cs, :], pg3[:, cs, :], pg3[:, cs, :], OP.mult)
                ve.tensor_reduce(msq[:, cs], pg2[:, cs, :], mybir.AxisListType.X, OP.add)
                act.activation(m2[:, cs], mean[:, cs], AF.Square)
                # var2 = msq - mean^2/D ; std = sqrt(max(var2,0)/(D-1))
                ve.scalar_tensor_tensor(out=var[:, cs], in0=m2[:, cs],
                                        scalar=float(-1.0 / D), in1=msq[:, cs],
                                        op0=OP.mult, op1=OP.add)
                ve.tensor_scalar(out=var[:, cs], in0=var[:, cs], scalar1=0.0,
                                 scalar2=None, op0=OP.max)
                act.activation(std[:, cs], var[:, cs], AF.Sqrt,
                               scale=float(1.0 / (D - 1)))
                ve.tensor_scalar(out=std[:, cs], in0=std[:, cs], scalar1=1e-6,
                                 scalar2=None, op0=OP.add)
                ve.reciprocal_approx_fast(istd[:, cs], std[:, cs])
                ve.scalar_tensor_tensor(out=negmistd[:, cs], in0=mean[:, cs],
                                        scalar=float(-1.0 / D),
                                        in1=istd[:, cs], op0=OP.mult, op1=OP.mult)
                istd_b = istd[:, cs].unsqueeze(2).broadcast_to([128, n, K])
                ve.tensor_tensor(pgi3[:, cs, :], pg3[:, cs, :], istd_b, OP.mult)
                pgi_b2 = pgi3[:, cs, :].unsqueeze(3).broadcast_to([128, n, K, 2])
                ve.tensor_copy(pgiX4[:, cs, :, :], pgi_b2)
                # pair-expanded negmistd (bf16) for the 2x add
                nm_b2 = negmistd[:, cs].unsqueeze(2).broadcast_to([128, n, 2])
                ve.tensor_copy(nmX3[:, cs, :], nm_b2)

            def final(jq):
                for q in range(4):
                    b = 4 * jq + q
                    x_ex = xep.tile([128, NPT], BF16, name="x_ex", tag="x_ex")
                    out_ex = op_.tile([128, NPT], BF16, name="out_ex", tag="out_ex")
                    # X = mask * pgi  (bf16)
                    if X_ENG[b] == 've':
                        # 4 per-k ops, all operands packed -> 2x_1p
                        xk = x_ex.rearrange("p (t k c2 w) -> p k t c2 w",
                                            t=T_PER_EX, k=K, w=2)
                        mk = masks5[:, jq, :, q, :] \
                            .rearrange("p t (c2 w) -> p t c2 w", w=2)
                        for k in range(K):
                            pk = pgiX4[:, 8 * b:8 * b + 8, k, :].unsqueeze(2) \
                                .broadcast_to([128, T_PER_EX, C // 2, 2])
                            ve.tensor_tensor(xk[:, k], mk, pk, OP.mult)
                    else:
                        x4v = x_ex.rearrange("p (t k c) -> p t k c",
                                             t=T_PER_EX, c=C)
                        mask_b = masks5[:, jq, :, q, :].unsqueeze(2) \
                            .broadcast_to([128, T_PER_EX, K, C])
                        pgi_b = pgi3[:, 8 * b:8 * b + 8, :].unsqueeze(3) \
                            .broadcast_to([128, T_PER_EX, K, C])
                        gp.tensor_tensor(x4v, mask_b, pgi_b, OP.mult)
                    # out = X + negmistd
                    if ADD_ENG[b] == 've':
                        ow = out_ex.rearrange("p (t f w) -> p t f w",
                                              t=T_PER_EX, w=2)
                        xw = x_ex.rearrange("p (t f w) -> p t f w",
                                            t=T_PER_EX, w=2)
                        nw = nmX3[:, 8 * b:8 * b + 8, :].unsqueeze(2) \
                            .broadcast_to([128, T_PER_EX, 64, 2])
                        ve.tensor_tensor(ow, xw, nw, OP.add)
                    else:
                        for t in range(T_PER_EX):
                            jg = 8 * b + t
                            act.activation(out_ex[:, D * t:D * (t + 1)],
                                           x_ex[:, D * t:D * (t + 1)],
                                           AF.Identity,
                                           bias=negmistd[:, jg:jg + 1], scale=1.0)
                    dst = out_d.rearrange("b (p t) j -> b p (t j)", p=128)[b]
                    sy.dma_start(dst, out_ex)

            preamble(0)
            preamble(1)
            stats_pair(0)
            preamble(2)
            preamble(3)
            stats_pair(1)
            for jp in range(4):
                if jp + 2 < 4:
                    preamble(2 * jp + 4)
                    preamble(2 * jp + 5)
                    stats_pair(jp + 2)
                final(2 * jp)
                final(2 * jp + 1)

    if not nc.is_finalized():
        nc.finalize()
    return nc


_NC = None


def _get_nc():
    global _NC
    if _NC is None:
        _NC = build_nc()
    return _NC


def make_in_maps(x: np.ndarray) -> list:
    """Host permute: xp[p, b, t, c] = x[b, 8p+t, c], per core."""
    x = np.ascontiguousarray(np.asarray(x, dtype=np.float32))
    B = x.shape[0]
    per = B // 8
    cf = _consts_f32()
    xp_all = x.reshape(B, 128, T_PER_EX, 4)
    return [
        {"x": np.ascontiguousarray(
            xp_all[i * per:(i + 1) * per].transpose(1, 0, 2, 3)
         ).reshape(128, per * T_PER_EX * 4),
         "cf": cf} for i in range(8)
    ]


def kernel(x: np.ndarray) -> np.ndarray:
    from concourse.bass_utils import run_bass_kernel_spmd

    n_cores = 8
    nc = _get_nc()
    in_maps = make_in_maps(x)
    res = run_bass_kernel_spmd(nc, in_maps, core_ids=list(range(n_cores)))
    return np.concatenate(
        [np.asarray(r["out"]).astype(np.float32) for r in res.results], axis=0)


if __name__ == "__main__":
    from concourse.bass_interp import CoreSim

    rng = np.random.default_rng(0)
    x = (rng.standard_normal((EX, NPT, 4)) * 2.0).astype(np.float32)
    x[..., 3] = rng.integers(0, C, size=(EX, NPT)).astype(np.float32)
    nc = build_nc()
    sim = CoreSim(nc)
    xp = np.ascontiguousarray(
        x.reshape(EX, 128, T_PER_EX, 4).transpose(1, 0, 2, 3)
    ).reshape(128, EX * T_PER_EX * 4)
    sim.tensor("x")[:] = xp
    sim.tensor("cf")[:] = _consts_f32()
    sim.simulate()
    got = np.array(sim.tensor("out")).astype(np.float32)

    xyz = x[..., :3]; clsf_ = x[..., 3]
    r = np.sqrt((xyz * xyz).sum(-1)); rinv = 1.0 / r
    RS = [0.0, 1.5, 3.0, 4.5]
    radial = np.exp(-ETA * (np.array(RS, np.float32)[None, None] - r[..., None]) ** 2)
    env = 0.5 * np.cos(np.pi * np.minimum(r, RC) / RC) + 0.5
    sh = np.stack([np.full_like(r, Y00), C1 * xyz[..., 1] * rinv,
                   C1 * xyz[..., 2] * rinv, C1 * xyz[..., 0] * rinv], -1)
    prod = sh * radial * env[..., None]
    onehot = (clsf_[..., None] == np.arange(C, dtype=np.float32)).astype(np.float32)
    pos = (prod[..., :, None] * onehot[..., None, :]).reshape(EX, NPT, D)
    norm = np.sqrt((pos * pos).sum(1, keepdims=True))
    pos = pos / np.maximum(norm, 1e-12)
    mean_ = pos.mean(-1, keepdims=True)
    std_ = pos.std(-1, ddof=1, keepdims=True)
    want = (pos - mean_) / (std_ + 1e-6)
    err = np.abs(got - want)
    print("sim absmax err:", err.max(), "ref absmax:", np.abs(want).max())
    rel = np.linalg.norm((got - want).ravel()) / np.linalg.norm(want.ravel())
    print("sim rel err:", rel)


# revision 12
# speedup vs baseline: 1.1570x; 1.1570x over previous
"""Trainium2 Bass kernel for nn_AtomicPositionalEncoding (v2).

kernel(**inputs): FULL x [256,1024,4] f32 -> FULL out [256,1024,128] f32.
Shards batch across 8 NeuronCores (32 examples each), one SPMD Bass program.

v2 design vs baseline:
- Point mapping n = 8*p + t (partition-major): input loads via direct
  strided DMA (no PE shuffle); output DMA per example is one 2KB
  contiguous chunk per partition -> full DMA efficiency at bf16.
- Output in bf16 (halves HBM write traffic); host casts back to f32.
- Epilogue per example: X = mask*pgi via 4 per-k tensor_tensor ops with
  all operands bf16 + packed pair views (DVE 2x_1p mode), then
  out = X + negmistd via one pair-packed 2x add (or per-tile act ops /
  gpsimd ops per the engine split tables).
- G gathered directly in point layout: G_A[pt,(q,k)] =
  matmul(weights=onehotT_tile, rhs=squad) -- no layout-B roundtrip.
- Stats computed per pair of quads; var folded as
  std = sqrt(max(msq - mean^2/D, 0)/(D-1)); fast approx reciprocals.

Column order: j = (b, t), b major: quad jq owns cols 32*jq..32*jq+31,
ordered (q, t) within the quad. masks/prodm2 stored t-major per quad
so hist/transpose slices are contiguous.
"""

import os
import sys

import numpy as np

for p in ("/opt/trn_rl_repo", "/root/.axon_site/_ro/trn_rl_repo"):
    if os.path.isdir(p) and p not in sys.path:
        sys.path.insert(0, p)

import concourse.bass as bass
import concourse.bacc as bacc
import concourse.mybir as mybir
from concourse.tile import TileContext

F32 = mybir.dt.float32
BF16 = mybir.dt.bfloat16

EX = 32          # examples per core
NPT = 1024       # points per example
T_PER_EX = 8     # 8 tiles of 128 points per example (t = n % 8)
NCOL = EX * T_PER_EX          # 256 point-tile columns
NQ = 8                        # quads of 4 examples
C = 32
K = 4
D = 128
ETA = 4.0
RC = 6.0
Y00 = 0.5 / np.sqrt(np.pi)
C1 = np.sqrt(3.0 / (4.0 * np.pi))

AF = mybir.ActivationFunctionType
OP = mybir.AluOpType

# ---- tunable engine splits ----
# X = mask*pgi expansion engine per example ('ve' = 4 per-k 2x ops, 'gp' = 1 op)
X_ENG = (['ve', 've', 've', 'gp'] * 8)
# out = X + negmistd engine per example ('ve' = 1 pair-packed 2x op,
# 'act' = 8 per-tile Identity+bias ops)
ADD_ENG = (['ve', 've', 'act', 've'] * 8)


def _consts_f32() -> np.ndarray:
    iota32 = np.tile(np.arange(C, dtype=np.float32), (128, 1))          # [128,32]
    blockmask = np.zeros((128, 16), dtype=np.float32)                   # [128,16]
    for pp_ in range(128):
        for f in range(16):
            if pp_ // 32 == f // 4:
                blockmask[pp_, f] = 1.0
    ident = np.eye(128, dtype=np.float32)                               # [128,128]
    bconst = np.tile(np.array([np.pi / 2, -1.5, -3.0, -4.5], np.float32), (128, 1))
    return np.concatenate(
        [iota32.ravel(), blockmask.ravel(), ident.ravel(), bconst.ravel()]
    )


CF_SIZES = [128 * 32, 128 * 16, 128 * 128, 128 * 4]
CF_TOTAL = sum(CF_SIZES)


def build_nc() -> bass.Bass:
    nc = bacc.Bacc()
    # x pre-permuted on host: xp[p, b, t, c] = x[b, 8p+t, c]
    x_d = nc.dram_tensor("x", [128, EX * T_PER_EX * 4], F32, kind="ExternalInput")
    cf_d = nc.dram_tensor("cf", [CF_TOTAL], F32, kind="ExternalInput")
    out_d = nc.dram_tensor("out", [EX, NPT, D], BF16, kind="ExternalOutput")

    with TileContext(nc) as tc:
        with (
            tc.tile_pool(name="persist", bufs=1) as pp,
            tc.tile_pool(name="ohp", bufs=4) as ohp,
            tc.tile_pool(name="outp", bufs=10) as op_,
            tc.tile_pool(name="xep", bufs=10) as xep,
            tc.tile_pool(name="ph", bufs=2, space="PSUM") as ph,       # hist
            tc.tile_pool(name="poh", bufs=3, space="PSUM") as poh,     # onehotT
            tc.tile_pool(name="pga", bufs=2, space="PSUM") as pga,     # G_A
        ):
            ve, act, gp, pe, sy = nc.vector, nc.scalar, nc.gpsimd, nc.tensor, nc.sync

            # ---- constants ----
            offs = np.cumsum([0] + CF_SIZES)
            def cslice(i, shape):
                t = pp.tile(shape, F32, name=f"const{i}", tag=f"const{i}")
                src = cf_d[offs[i]:offs[i + 1]].rearrange("(p f) -> p f", p=shape[0])
                sy.dma_start(t, src)
                return t
            iota32 = cslice(0, [128, 32])
            blockmask = cslice(1, [128, 16])
            ident = cslice(2, [128, 128])
            bconst = cslice(3, [128, 4])
            ident16 = pp.tile([128, 128], BF16, name="ident16", tag="ident16")
            ve.tensor_copy(ident16, ident)

            # ---- load x (host pre-permuted to (p, (b t c))) ----
            x_sb = pp.tile([128, NCOL * 4], F32, name="x", tag="x")
            sy.dma_start(x_sb, x_d[:, :])
            x4 = x_sb.rearrange("p (j c) -> p j c", c=4)
            xyz = x4[:, :, 0:3]
            clsf2 = x4[:, :, 3:4].rearrange("p j one -> p (j one)")   # [128,256]

            # ---- stage 1 (per point) ----
            sq = pp.tile([128, NCOL * 3], F32, name="sq", tag="sq").rearrange("p (j c) -> p j c", c=3)
            ve.tensor_tensor(sq, xyz, xyz, OP.mult)
            r2 = pp.tile([128, NCOL], F32, name="r2", tag="r2")
            ve.tensor_reduce(r2, sq, mybir.AxisListType.X, OP.add)
            r = pp.tile([128, NCOL], F32, name="r", tag="r")
            act.activation(r, r2, AF.Sqrt)
            rinv = pp.tile([128, NCOL], F32, name="rinv", tag="rinv")
            ve.reciprocal_approx_fast(rinv, r)
            rmin = pp.tile([128, NCOL], F32, name="rmin", tag="rmin")
            ve.tensor_scalar(out=rmin, in0=r, scalar1=float(RC), scalar2=None, op0=OP.min)
            cosv = pp.tile([128, NCOL], F32, name="cosv", tag="cosv")
            act.activation(cosv, rmin, AF.Sin, bias=bconst[:, 0:1], scale=float(-np.pi / RC))
            env = pp.tile([128, NCOL], F32, name="env", tag="env")
            ve.tensor_scalar(out=env, in0=cosv, scalar1=0.5, scalar2=0.5, op0=OP.mult, op1=OP.add)

            # group Squares then Exps to avoid act-table thrash; Exp in place
            radial = pp.tile([128, NCOL * K], F32, name="radial", tag="radial").rearrange("p (j k) -> p j k", k=K)
            for k in range(K):
                bias_k = 0.0 if k == 0 else bconst[:, k:k + 1]
                act.activation(radial[:, :, k:k + 1].rearrange("p j one -> p (j one)"),
                               r, AF.Square, bias=bias_k, scale=1.0)
            for k in range(K):
                rk = radial[:, :, k:k + 1].rearrange("p j one -> p (j one)")
                act.activation(rk, rk, AF.Exp, scale=float(-ETA))
            m = pp.tile([128, NCOL * K], F32, name="m", tag="m").rearrange("p (j k) -> p j k", k=K)
            env_b = env.unsqueeze(2).broadcast_to([128, NCOL, K])
            ve.tensor_tensor(m, radial, env_b, OP.mult)

            u3 = pp.tile([128, NCOL * 3], F32, name="u3", tag="u3").rearrange("p (j c) -> p j c", c=3)
            rinv_b = rinv.unsqueeze(2).broadcast_to([128, NCOL, 3])
            ve.tensor_tensor(u3, xyz, rinv_b, OP.mult)
            prodm = pp.tile([128, NCOL * K], F32, name="prodm", tag="prodm").rearrange("p (j k) -> p j k", k=K)
            ve.tensor_scalar(out=prodm[:, :, 0:1], in0=m[:, :, 0:1], scalar1=float(Y00),
                             scalar2=None, op0=OP.mult)
            ve.scalar_tensor_tensor(out=prodm[:, :, 1:3], in0=m[:, :, 1:3], scalar=float(C1),
                                    in1=u3[:, :, 1:3], op0=OP.mult, op1=OP.mult)
            ve.scalar_tensor_tensor(out=prodm[:, :, 3:4], in0=m[:, :, 3:4], scalar=float(C1),
                                    in1=u3[:, :, 0:1], op0=OP.mult, op1=OP.mult)
            # prodm^2 in bf16, t-major per quad: [128, (jq, t, q, k)]
            prodm2q = pp.tile([128, NCOL * K], BF16, name="prodm2", tag="prodm2") \
                        .rearrange("p (jq t q k) -> p jq t q k", jq=NQ, t=T_PER_EX, q=4)

            # ---- masks: onehot(cls) bf16, storage [128, (jq, t, q, c)] ----
            masks_flat = pp.tile([128, NCOL * C], BF16, name="masks", tag="masks")
            masks5 = masks_flat.rearrange("p (jq t q c) -> p jq t q c",
                                          jq=NQ, t=T_PER_EX, q=4, c=C)
            # bf16 iota + pair-expanded cls so is_equal runs in 2x mode
            iota16 = pp.tile([128, C], BF16, name="iota16", tag="iota16")
            ve.tensor_copy(iota16, iota32)
            iota16w = iota16.rearrange("p (c2 w) -> p c2 w", w=2)
            clsX = pp.tile([128, NCOL * 2], BF16, name="clsX", tag="clsX")
            clsX3 = clsX.rearrange("p (j two) -> p j two", two=2)
            ve.tensor_copy(clsX3, clsf2.unsqueeze(2).broadcast_to([128, NCOL, 2]))

            # ---- persistent tensors for the quad pipeline ----
            squadh = pp.tile([128, NQ * 16], BF16, name="squadh", tag="squadh")
            g_all = pp.tile([128, NQ * 128], F32, name="g_all", tag="g_all")
            pg = pp.tile([128, NCOL * K], F32, name="pg", tag="pg")
            pg3 = pg.rearrange("p (j k) -> p j k", k=K)
            pg2 = pp.tile([128, NCOL * K], F32, name="pg2", tag="pg2").rearrange("p (j k) -> p j k", k=K)
            pgi3 = pp.tile([128, NCOL * K], F32, name="pgi", tag="pgi").rearrange("p (j k) -> p j k", k=K)
            # pair-expanded pgi (bf16) for the 2x per-k expansion:
            # pgiX[p, j, k, w] = pgi[p, j, k], w in {0,1}
            pgiX = pp.tile([128, NCOL * K * 2], BF16, name="pgiX", tag="pgiX")
            pgiX4 = pgiX.rearrange("p (j k w) -> p j k w", k=K, w=2)
            mean = pp.tile([128, NCOL], F32, name="mean", tag="mean")
            msq = pp.tile([128, NCOL], F32, name="msq", tag="msq")
            m2 = pp.tile([128, NCOL], F32, name="m2", tag="m2")
            var = pp.tile([128, NCOL], F32, name="var", tag="var")
            std = pp.tile([128, NCOL], F32, name="std", tag="std")
            istd = pp.tile([128, NCOL], F32, name="istd", tag="istd")
            negmistd = pp.tile([128, NCOL], F32, name="negmistd", tag="negmistd")
            # pair-expanded negmistd for the 4x fill: [128, (j, 2)] bf16
            nmX = pp.tile([128, NCOL * 2], BF16, name="nmX", tag="nmX")
            nmX3 = nmX.rearrange("p (j two) -> p j two", two=2)

            # ---- per-quad pipeline with skew ----
            def preamble(jq):
                cs = slice(32 * jq, 32 * (jq + 1))        # this quad's 32 cols
                for q in range(4):
                    b = 4 * jq + q
                    # masks in 2x mode: all operands bf16, innermost packed pair
                    cls_w = clsX3[:, 8 * b:8 * b + 8, :].unsqueeze(2) \
                        .broadcast_to([128, T_PER_EX, C // 2, 2])
                    iota_w = iota16w.unsqueeze(1) \
                        .broadcast_to([128, T_PER_EX, C // 2, 2])
                    mq = masks5[:, jq, :, q, :] \
                        .rearrange("p t (c2 w) -> p t c2 w", w=2)
                    ve.tensor_tensor(mq, cls_w, iota_w, OP.is_equal)
                # prodm^2 for the whole quad in one op (strided view)
                pq = prodm[:, 32 * jq:32 * (jq + 1), :] \
                    .rearrange("p (q t) k -> p t q k", q=4)
                ve.tensor_tensor(prodm2q[:, jq], pq, pq, OP.mult)
                # hist[(q,c),(q',k)]: diag blocks = per-example hists
                hist_ps = ph.tile([128, 16], F32, name="hist_ps", tag="hist_ps")
                for t in range(T_PER_EX):
                    pe.matmul(hist_ps,
                              masks5[:, jq, t].rearrange("p a b -> p (a b)"),
                              prodm2q[:, jq, t].rearrange("p a b -> p (a b)"),
                              start=(t == 0), stop=(t == T_PER_EX - 1))
                # scale = 1/max(sqrt(hist),1e-12); off-diag junk masked by blockmask
                sr4 = pp.tile([128, 16], F32, name=f"sr4_{jq}", tag=f"sr4_{jq}")
                act.activation(sr4, hist_ps, AF.Sqrt)
                ve.tensor_scalar(out=sr4, in0=sr4, scalar1=1e-12, scalar2=None, op0=OP.max)
                ve.reciprocal_approx_fast(sr4, sr4)
                ve.tensor_tensor(squadh[:, 16 * jq:16 * (jq + 1)], sr4, blockmask, OP.mult)

                # onehotT tiles (PE transpose -> PSUM -> SBUF copy)
                oh_ps = poh.tile([128, NPT], BF16, name="oh_ps", tag="oh_ps")
                for t in range(T_PER_EX):
                    pe.transpose(oh_ps[:, 128 * t:128 * (t + 1)],
                                 masks5[:, jq, t].rearrange("p a b -> p (a b)"),
                                 ident16)
                oh_sb = ohp.tile([128, NPT], BF16, name="oh_sb", tag="oh_sb")
                if jq % 2 == 0:
                    ve.tensor_copy(oh_sb, oh_ps)
                else:
                    act.copy(oh_sb, oh_ps)
                # G_A gather: per t, [128 pt, 16 (q,k)] = ohT_t @ squad
                ga_ps = pga.tile([128, 128], F32, name="ga_ps", tag="ga_ps")
                for t in range(T_PER_EX):
                    pe.matmul(ga_ps[:, 16 * t:16 * (t + 1)],
                              oh_sb[:, 128 * t:128 * (t + 1)],
                              squadh[:, 16 * jq:16 * (jq + 1)],
                              start=True, stop=True)
                # reorder (t,q,k) -> (q,t,k) into g_all
                ga4 = ga_ps.rearrange("p (t q k) -> p q t k", t=T_PER_EX, q=4)
                g4 = g_all[:, 128 * jq:128 * (jq + 1)] \
                    .rearrange("p (q t k) -> p q t k", q=4, t=T_PER_EX)
                ve.tensor_copy(g4, ga4)
            def stats_pair(jp):
                """Per-point normalization scalars for quads 2jp, 2jp+1
                (64 cols at once to halve DVE op count)."""
                cs = slice(64 * jp, 64 * (jp + 1))
                n = 64
                gq3 = g_all[:, 256 * jp:256 * (jp + 1)] \
                    .rearrange("p (j k) -> p j k", k=K)
                ve.tensor_tensor(pg3[:, cs, :], prodm[:, cs, :], gq3, OP.mult)
                ve.tensor_reduce(mean[:, cs], pg3[:, cs, :], mybir.AxisListType.X, OP.add)
                gp.tensor_tensor(pg2[:, cs, :], pg3[:, cs, :], pg3[:, cs, :], OP.mult)
                ve.tensor_reduce(msq[:, cs], pg2[:, cs, :], mybir.AxisListType.X, OP.add)
                act.activation(m2[:, cs], mean[:, cs], AF.Square)
                # var2 = msq - mean^2/D ; std = sqrt(max(var2,0)/(D-1))
                ve.scalar_tensor_tensor(out=var[:, cs], in0=m2[:, cs],
                                        scalar=float(-1.0 / D), in1=msq[:, cs],
                                        op0=OP.mult, op1=OP.add)
                ve.tensor_scalar(out=var[:, cs], in0=var[:, cs], scalar1=0.0,
                                 scalar2=None, op0=OP.max)
                act.activation(std[:, cs], var[:, cs], AF.Sqrt,
                               scale=float(1.0 / (D - 1)))
                ve.tensor_scalar(out=std[:, cs], in0=std[:, cs], scalar1=1e-6,
                                 scalar2=None, op0=OP.add)
                ve.reciprocal_approx_fast(istd[:, cs], std[:, cs])
                ve.scalar_tensor_tensor(out=negmistd[:, cs], in0=mean[:, cs],
                                        scalar=float(-1.0 / D),
                                        in1=istd[:, cs], op0=OP.mult, op1=OP.mult)
                istd_b = istd[:, cs].unsqueeze(2).broadcast_to([128, n, K])
                ve.tensor_tensor(pgi3[:, cs, :], pg3[:, cs, :], istd_b, OP.mult)
                pgi_b2 = pgi3[:, cs, :].unsqueeze(3).broadcast_to([128, n, K, 2])
                ve.tensor_copy(pgiX4[:, cs, :, :], pgi_b2)
                # pair-expanded negmistd (bf16) for the 2x add
                nm_b2 = negmistd[:, cs].unsqueeze(2).broadcast_to([128, n, 2])
                ve.tensor_copy(nmX3[:, cs, :], nm_b2)

            def final(jq):
                for q in range(4):
                    b = 4 * jq + q
                    x_ex = xep.tile([128, NPT], BF16, name="x_ex", tag="x_ex")
                    out_ex = op_.tile([128, NPT], BF16, name="out_ex", tag="out_ex")
                    # X = mask * pgi  (bf16)
                    if X_ENG[b] == 've':
                        # 4 per-k ops, all operands packed -> 2x_1p
                        xk = x_ex.rearrange("p (t k c2 w) -> p k t c2 w",
                                            t=T_PER_EX, k=K, w=2)
                        mk = masks5[:, jq, :, q, :] \
                            .rearrange("p t (c2 w) -> p t c2 w", w=2)
                        for k in range(K):
                            pk = pgiX4[:, 8 * b:8 * b + 8, k, :].unsqueeze(2) \
                                .broadcast_to([128, T_PER_EX, C // 2, 2])
                            ve.tensor_tensor(xk[:, k], mk, pk, OP.mult)
                    else:
                        x4v = x_ex.rearrange("p (t k c) -> p t k c",
                                             t=T_PER_EX, c=C)
                        mask_b = masks5[:, jq, :, q, :].unsqueeze(2) \
                            .broadcast_to([128, T_PER_EX, K, C])
                        pgi_b = pgi3[:, 8 * b:8 * b + 8, :].unsqueeze(3) \
                            .broadcast_to([128, T_PER_EX, K, C])
                        gp.tensor_tensor(x4v, mask_b, pgi_b, OP.mult)
                    # out = X + negmistd
                    if ADD_ENG[b] == 've':
                        ow = out_ex.rearrange("p (t f w) -> p t f w",
                                              t=T_PER_EX, w=2)
                        xw = x_ex.rearrange("p (t f w) -> p t f w",
                                            t=T_PER_EX, w=2)
                        nw = nmX3[:, 8 * b:8 * b + 8, :].unsqueeze(2) \
                            .broadcast_to([128, T_PER_EX, 64, 2])
                        ve.tensor_tensor(ow, xw, nw, OP.add)
                    else:
                        for t in range(T_PER_EX):
                            jg = 8 * b + t
                            act.activation(out_ex[:, D * t:D * (t + 1)],
                                           x_ex[:, D * t:D * (t + 1)],
                                           AF.Identity,
                                           bias=negmistd[:, jg:jg + 1], scale=1.0)
                    dst = out_d.rearrange("b (p t) j -> b p (t j)", p=128)[b]
                    sy.dma_start(dst, out_ex)

            preamble(0)
            preamble(1)
            stats_pair(0)
            preamble(2)
            preamble(3)
            stats_pair(1)
            for jp in range(4):
                if jp + 2 < 4:
                    preamble(2 * jp + 4)
                    preamble(2 * jp + 5)
                    stats_pair(jp + 2)
                final(2 * jp)
                final(2 * jp + 1)

    if not nc.is_finalized():
        nc.finalize()
    return nc


_NC = None


def _get_nc():
    global _NC
    if _NC is None:
        _NC = build_nc()
    return _NC


def make_in_maps(x: np.ndarray) -> list:
    """Host permute: xp[p, b, t, c] = x[b, 8p+t, c], per core."""
    x = np.ascontiguousarray(np.asarray(x, dtype=np.float32))
    B = x.shape[0]
    per = B // 8
    cf = _consts_f32()
    xp_all = x.reshape(B, 128, T_PER_EX, 4)
    return [
        {"x": np.ascontiguousarray(
            xp_all[i * per:(i + 1) * per].transpose(1, 0, 2, 3)
         ).reshape(128, per * T_PER_EX * 4),
         "cf": cf} for i in range(8)
    ]


def kernel(x: np.ndarray) -> np.ndarray:
    from concourse.bass_utils import run_bass_kernel_spmd

    n_cores = 8
    nc = _get_nc()
    in_maps = make_in_maps(x)
    res = run_bass_kernel_spmd(nc, in_maps, core_ids=list(range(n_cores)))
    return np.concatenate(
        [np.asarray(r["out"]).astype(np.float32) for r in res.results], axis=0)


if __name__ == "__main__":
    from concourse.bass_interp import CoreSim

    rng = np.random.default_rng(0)
    x = (rng.standard_normal((EX, NPT, 4)) * 2.0).astype(np.float32)
    x[..., 3] = rng.integers(0, C, size=(EX, NPT)).astype(np.float32)
    nc = build_nc()
    sim = CoreSim(nc)
    xp = np.ascontiguousarray(
        x.reshape(EX, 128, T_PER_EX, 4).transpose(1, 0, 2, 3)
    ).reshape(128, EX * T_PER_EX * 4)
    sim.tensor("x")[:] = xp
    sim.tensor("cf")[:] = _consts_f32()
    sim.simulate()
    got = np.array(sim.tensor("out")).astype(np.float32)

    xyz = x[..., :3]; clsf_ = x[..., 3]
    r = np.sqrt((xyz * xyz).sum(-1)); rinv = 1.0 / r
    RS = [0.0, 1.5, 3.0, 4.5]
    radial = np.exp(-ETA * (np.array(RS, np.float32)[None, None] - r[..., None]) ** 2)
    env = 0.5 * np.cos(np.pi * np.minimum(r, RC) / RC) + 0.5
    sh = np.stack([np.full_like(r, Y00), C1 * xyz[..., 1] * rinv,
                   C1 * xyz[..., 2] * rinv, C1 * xyz[..., 0] * rinv], -1)
    prod = sh * radial * env[..., None]
    onehot = (clsf_[..., None] == np.arange(C, dtype=np.float32)).astype(np.float32)
    pos = (prod[..., :, None] * onehot[..., None, :]).reshape(EX, NPT, D)
    norm = np.sqrt((pos * pos).sum(1, keepdims=True))
    pos = pos / np.maximum(norm, 1e-12)
    mean_ = pos.mean(-1, keepdims=True)
    std_ = pos.std(-1, ddof=1, keepdims=True)
    want = (pos - mean_) / (std_ + 1e-6)
    err = np.abs(got - want)
    print("sim absmax err:", err.max(), "ref absmax:", np.abs(want).max())
    rel = np.linalg.norm((got - want).ravel()) / np.linalg.norm(want.ravel())
    print("sim rel err:", rel)


# revision 16
# speedup vs baseline: 1.1742x; 1.0148x over previous
"""Trainium2 Bass kernel for nn_AtomicPositionalEncoding (v2).

kernel(**inputs): FULL x [256,1024,4] f32 -> FULL out [256,1024,128] f32.
Shards batch across 8 NeuronCores (32 examples each), one SPMD Bass program.

v2 design vs baseline:
- Point mapping n = 8*p + t (partition-major): input loads via direct
  strided DMA (no PE shuffle); output DMA per example is one 2KB
  contiguous chunk per partition -> full DMA efficiency at bf16.
- Output in bf16 (halves HBM write traffic); host casts back to f32.
- Epilogue per example: X = mask*pgi via 4 per-k tensor_tensor ops with
  all operands bf16 + packed pair views (DVE 2x_1p mode), then
  out = X + negmistd via one pair-packed 2x add (or per-tile act ops /
  gpsimd ops per the engine split tables).
- G gathered directly in point layout: G_A[pt,(q,k)] =
  matmul(weights=onehotT_tile, rhs=squad) -- no layout-B roundtrip.
- Stats computed per pair of quads; var folded as
  std = sqrt(max(msq - mean^2/D, 0)/(D-1)); fast approx reciprocals.

Column order: j = (b, t), b major: quad jq owns cols 32*jq..32*jq+31,
ordered (q, t) within the quad. masks/prodm2 stored t-major per quad
so hist/transpose slices are contiguous.
"""

import os
import sys

import numpy as np

for p in ("/opt/trn_rl_repo", "/root/.axon_site/_ro/trn_rl_repo"):
    if os.path.isdir(p) and p not in sys.path:
        sys.path.insert(0, p)

import concourse.bass as bass
import concourse.bacc as bacc
import concourse.mybir as mybir
from concourse.tile import TileContext

F32 = mybir.dt.float32
BF16 = mybir.dt.bfloat16

EX = 32          # examples per core
NPT = 1024       # points per example
T_PER_EX = 8     # 8 tiles of 128 points per example (t = n % 8)
NCOL = EX * T_PER_EX          # 256 point-tile columns
NQ = 8                        # quads of 4 examples
C = 32
K = 4
D = 128
ETA = 4.0
RC = 6.0
Y00 = 0.5 / np.sqrt(np.pi)
C1 = np.sqrt(3.0 / (4.0 * np.pi))

AF = mybir.ActivationFunctionType
OP = mybir.AluOpType

# ---- tunable engine splits ----
# X = mask*pgi expansion engine per example ('ve' = 4 per-k 2x ops, 'gp' = 1 op)
X_ENG = (['ve', 've', 've', 'gp'] * 8)
# out = X + negmistd engine per example ('ve' = 1 pair-packed 2x op,
# 'act' = 8 per-tile Identity+bias ops)
ADD_ENG = (['ve', 've', 'act', 've'] * 8)


def _consts_f32() -> np.ndarray:
    iota32 = np.tile(np.arange(C, dtype=np.float32), (128, 1))          # [128,32]
    blockmask = np.zeros((128, 16), dtype=np.float32)                   # [128,16]
    for pp_ in range(128):
        for f in range(16):
            if pp_ // 32 == f // 4:
                blockmask[pp_, f] = 1.0
    ident = np.eye(128, dtype=np.float32)                               # [128,128]
    bconst = np.tile(np.array([np.pi / 2, -1.5, -3.0, -4.5], np.float32), (128, 1))
    return np.concatenate(
        [iota32.ravel(), blockmask.ravel(), ident.ravel(), bconst.ravel()]
    )


CF_SIZES = [128 * 32, 128 * 16, 128 * 128, 128 * 4]
CF_TOTAL = sum(CF_SIZES)


def build_nc() -> bass.Bass:
    nc = bacc.Bacc()
    # x pre-permuted on host: xp[p, b, t, c] = x[b, 8p+t, c]
    x_d = nc.dram_tensor("x", [128, EX * T_PER_EX * 4], F32, kind="ExternalInput")
    cf_d = nc.dram_tensor("cf", [CF_TOTAL], F32, kind="ExternalInput")
    out_d = nc.dram_tensor("out", [EX, NPT, D], BF16, kind="ExternalOutput")

    with TileContext(nc) as tc:
        with (
            tc.tile_pool(name="persist", bufs=1) as pp,
            tc.tile_pool(name="ohp", bufs=3) as ohp,
            tc.tile_pool(name="outp", bufs=6) as op_,
            tc.tile_pool(name="xep", bufs=6) as xep,
            tc.tile_pool(name="ph", bufs=2, space="PSUM") as ph,       # hist
            tc.tile_pool(name="poh", bufs=2, space="PSUM") as poh,     # onehotT
            tc.tile_pool(name="pga", bufs=2, space="PSUM") as pga,     # G_A
        ):
            ve, act, gp, pe, sy = nc.vector, nc.scalar, nc.gpsimd, nc.tensor, nc.sync

            # ---- constants ----
            offs = np.cumsum([0] + CF_SIZES)
            def cslice(i, shape):
                t = pp.tile(shape, F32, name=f"const{i}", tag=f"const{i}")
                src = cf_d[offs[i]:offs[i + 1]].rearrange("(p f) -> p f", p=shape[0])
                sy.dma_start(t, src)
                return t
            iota32 = cslice(0, [128, 32])
            blockmask = cslice(1, [128, 16])
            ident = cslice(2, [128, 128])
            bconst = cslice(3, [128, 4])
            ident16 = pp.tile([128, 128], BF16, name="ident16", tag="ident16")
            ve.tensor_copy(ident16, ident)

            # ---- load x (host pre-permuted to (p, (b t c))) ----
            x_sb = pp.tile([128, NCOL * 4], F32, name="x", tag="x")
            sy.dma_start(x_sb, x_d[:, :])
            x4 = x_sb.rearrange("p (j c) -> p j c", c=4)
            xyz = x4[:, :, 0:3]
            clsf2 = x4[:, :, 3:4].rearrange("p j one -> p (j one)")   # [128,256]

            # ---- masks decls (emitted early to fill stage-1 DVE stalls) ----
            masks_flat = pp.tile([128, NCOL * C], BF16, name="masks", tag="masks")
            masks5 = masks_flat.rearrange("p (jq t q c) -> p jq t q c",
                                          jq=NQ, t=T_PER_EX, q=4, c=C)
            iota16 = pp.tile([128, C], BF16, name="iota16", tag="iota16")
            ve.tensor_copy(iota16, iota32)
            iota16w = iota16.rearrange("p (c2 w) -> p c2 w", w=2)
            clsX = pp.tile([128, NCOL * 2], BF16, name="clsX", tag="clsX")
            clsX3 = clsX.rearrange("p (j two) -> p j two", two=2)
            ve.tensor_copy(clsX3, clsf2.unsqueeze(2).broadcast_to([128, NCOL, 2]))

            def emit_masks(jq):
                """onehot(cls) for one quad; 2x mode via bf16 packed pairs."""
                for q in range(4):
                    b = 4 * jq + q
                    cls_w = clsX3[:, 8 * b:8 * b + 8, :].unsqueeze(2) \
                        .broadcast_to([128, T_PER_EX, C // 2, 2])
                    iota_w = iota16w.unsqueeze(1) \
                        .broadcast_to([128, T_PER_EX, C // 2, 2])
                    mq = masks5[:, jq, :, q, :] \
                        .rearrange("p t (c2 w) -> p t c2 w", w=2)
                    ve.tensor_tensor(mq, cls_w, iota_w, OP.is_equal)

            # ---- stage 1 (per point) ----
            sq = pp.tile([128, NCOL * 3], F32, name="sq", tag="sq").rearrange("p (j c) -> p j c", c=3)
            ve.tensor_tensor(sq, xyz, xyz, OP.mult)
            r2 = pp.tile([128, NCOL], F32, name="r2", tag="r2")
            ve.tensor_reduce(r2, sq, mybir.AxisListType.X, OP.add)
            r = pp.tile([128, NCOL], F32, name="r", tag="r")
            act.activation(r, r2, AF.Sqrt)
            emit_masks(0)                 # DVE fills the sqrt round-trip
            rinv = pp.tile([128, NCOL], F32, name="rinv", tag="rinv")
            ve.reciprocal_approx_fast(rinv, r)
            rmin = pp.tile([128, NCOL], F32, name="rmin", tag="rmin")
            ve.tensor_scalar(out=rmin, in0=r, scalar1=float(RC), scalar2=None, op0=OP.min)
            cosv = pp.tile([128, NCOL], F32, name="cosv", tag="cosv")
            act.activation(cosv, rmin, AF.Sin, bias=bconst[:, 0:1], scale=float(-np.pi / RC))
            emit_masks(1)                 # DVE fills the sin round-trip
            env = pp.tile([128, NCOL], F32, name="env", tag="env")
            ve.tensor_scalar(out=env, in0=cosv, scalar1=0.5, scalar2=0.5, op0=OP.mult, op1=OP.add)

            # group Squares then Exps to avoid act-table thrash; Exp in place
            radial = pp.tile([128, NCOL * K], F32, name="radial", tag="radial").rearrange("p (j k) -> p j k", k=K)
            for k in range(K):
                bias_k = 0.0 if k == 0 else bconst[:, k:k + 1]
                act.activation(radial[:, :, k:k + 1].rearrange("p j one -> p (j one)"),
                               r, AF.Square, bias=bias_k, scale=1.0)
            emit_masks(2)                 # DVE fills the square/exp stretch
            emit_masks(3)
            for k in range(K):
                rk = radial[:, :, k:k + 1].rearrange("p j one -> p (j one)")
                act.activation(rk, rk, AF.Exp, scale=float(-ETA))
            emit_masks(4)
            m = pp.tile([128, NCOL * K], F32, name="m", tag="m").rearrange("p (j k) -> p j k", k=K)
            env_b = env.unsqueeze(2).broadcast_to([128, NCOL, K])
            ve.tensor_tensor(m, radial, env_b, OP.mult)

            u3 = pp.tile([128, NCOL * 3], F32, name="u3", tag="u3").rearrange("p (j c) -> p j c", c=3)
            rinv_b = rinv.unsqueeze(2).broadcast_to([128, NCOL, 3])
            ve.tensor_tensor(u3, xyz, rinv_b, OP.mult)
            prodm = pp.tile([128, NCOL * K], F32, name="prodm", tag="prodm").rearrange("p (j k) -> p j k", k=K)
            ve.tensor_scalar(out=prodm[:, :, 0:1], in0=m[:, :, 0:1], scalar1=float(Y00),
                             scalar2=None, op0=OP.mult)
            ve.scalar_tensor_tensor(out=prodm[:, :, 1:3], in0=m[:, :, 1:3], scalar=float(C1),
                                    in1=u3[:, :, 1:3], op0=OP.mult, op1=OP.mult)
            ve.scalar_tensor_tensor(out=prodm[:, :, 3:4], in0=m[:, :, 3:4], scalar=float(C1),
                                    in1=u3[:, :, 0:1], op0=OP.mult, op1=OP.mult)
            # prodm^2 in bf16, t-major per quad: [128, (jq, t, q, k)]
            prodm2q = pp.tile([128, NCOL * K], BF16, name="prodm2", tag="prodm2") \
                        .rearrange("p (jq t q k) -> p jq t q k", jq=NQ, t=T_PER_EX, q=4)

            # ---- persistent tensors for the quad pipeline ----
            squadh = pp.tile([128, NQ * 16], BF16, name="squadh", tag="squadh")
            g_all = pp.tile([128, NQ * 128], F32, name="g_all", tag="g_all")
            pg = pp.tile([128, NCOL * K], F32, name="pg", tag="pg")
            pg3 = pg.rearrange("p (j k) -> p j k", k=K)
            pg2 = pp.tile([128, NCOL * K], F32, name="pg2", tag="pg2").rearrange("p (j k) -> p j k", k=K)
            pgi3 = pp.tile([128, NCOL * K], F32, name="pgi", tag="pgi").rearrange("p (j k) -> p j k", k=K)
            # pair-expanded pgi (bf16) for the 2x per-k expansion:
            # pgiX[p, j, k, w] = pgi[p, j, k], w in {0,1}
            pgiX = pp.tile([128, NCOL * K * 2], BF16, name="pgiX", tag="pgiX")
            pgiX4 = pgiX.rearrange("p (j k w) -> p j k w", k=K, w=2)
            mean = pp.tile([128, NCOL], F32, name="mean", tag="mean")
            msq = pp.tile([128, NCOL], F32, name="msq", tag="msq")
            m2 = pp.tile([128, NCOL], F32, name="m2", tag="m2")
            var = pp.tile([128, NCOL], F32, name="var", tag="var")
            std = pp.tile([128, NCOL], F32, name="std", tag="std")
            istd = pp.tile([128, NCOL], F32, name="istd", tag="istd")
            negmistd = pp.tile([128, NCOL], F32, name="negmistd", tag="negmistd")
            # pair-expanded negmistd for the 4x fill: [128, (j, 2)] bf16
            nmX = pp.tile([128, NCOL * 2], BF16, name="nmX", tag="nmX")
            nmX3 = nmX.rearrange("p (j two) -> p j two", two=2)

            # ---- per-quad pipeline with skew ----
            def preamble(jq):
                cs = slice(32 * jq, 32 * (jq + 1))        # this quad's 32 cols
                if jq >= 5:
                    emit_masks(jq)        # quads 0-4 emitted during stage 1
                # prodm^2 for the whole quad in one op (strided view)
                pq = prodm[:, 32 * jq:32 * (jq + 1), :] \
                    .rearrange("p (q t) k -> p t q k", q=4)
                ve.tensor_tensor(prodm2q[:, jq], pq, pq, OP.mult)
                # hist[(q,c),(q',k)]: diag blocks = per-example hists
                hist_ps = ph.tile([128, 16], F32, name="hist_ps", tag="hist_ps")
                for t in range(T_PER_EX):
                    pe.matmul(hist_ps,
                              masks5[:, jq, t].rearrange("p a b -> p (a b)"),
                              prodm2q[:, jq, t].rearrange("p a b -> p (a b)"),
                              start=(t == 0), stop=(t == T_PER_EX - 1))
                # scale = 1/max(sqrt(hist),1e-12); off-diag junk masked by blockmask
                sr4 = pp.tile([128, 16], F32, name=f"sr4_{jq}", tag=f"sr4_{jq}")
                act.activation(sr4, hist_ps, AF.Sqrt)
                ve.tensor_scalar(out=sr4, in0=sr4, scalar1=1e-12, scalar2=None, op0=OP.max)
                ve.reciprocal_approx_fast(sr4, sr4)
                ve.tensor_tensor(squadh[:, 16 * jq:16 * (jq + 1)], sr4, blockmask, OP.mult)

                # onehotT tiles (PE transpose -> PSUM -> SBUF copy)
                oh_ps = poh.tile([128, NPT], BF16, name="oh_ps", tag="oh_ps")
                for t in range(T_PER_EX):
                    pe.transpose(oh_ps[:, 128 * t:128 * (t + 1)],
                                 masks5[:, jq, t].rearrange("p a b -> p (a b)"),
                                 ident16)
                oh_sb = ohp.tile([128, NPT], BF16, name="oh_sb", tag="oh_sb")
                if jq % 2 == 0:
                    ve.tensor_copy(oh_sb, oh_ps)
                else:
                    act.copy(oh_sb, oh_ps)
                # G_A gather: per t, [128 pt, 16 (q,k)] = ohT_t @ squad
                ga_ps = pga.tile([128, 128], F32, name="ga_ps", tag="ga_ps")
                for t in range(T_PER_EX):
                    pe.matmul(ga_ps[:, 16 * t:16 * (t + 1)],
                              oh_sb[:, 128 * t:128 * (t + 1)],
                              squadh[:, 16 * jq:16 * (jq + 1)],
                              start=True, stop=True)
                # reorder (t,q,k) -> (q,t,k) into g_all
                ga4 = ga_ps.rearrange("p (t q k) -> p q t k", t=T_PER_EX, q=4)
                g4 = g_all[:, 128 * jq:128 * (jq + 1)] \
                    .rearrange("p (q t k) -> p q t k", q=4, t=T_PER_EX)
                ve.tensor_copy(g4, ga4)
            def stats_pair(jp):
                """Per-point normalization scalars for quads 2jp, 2jp+1
                (64 cols at once to halve DVE op count)."""
                cs = slice(64 * jp, 64 * (jp + 1))
                n = 64
                gq3 = g_all[:, 256 * jp:256 * (jp + 1)] \
                    .rearrange("p (j k) -> p j k", k=K)
                ve.tensor_tensor(pg3[:, cs, :], prodm[:, cs, :], gq3, OP.mult)
                ve.tensor_reduce(mean[:, cs], pg3[:, cs, :], mybir.AxisListType.X, OP.add)
                gp.tensor_tensor(pg2[:, cs, :], pg3[:, cs, :], pg3[:, cs, :], OP.mult)
                ve.tensor_reduce(msq[:, cs], pg2[:, cs, :], mybir.AxisListType.X, OP.add)
                act.activation(m2[:, cs], mean[:, cs], AF.Square)
                # var2 = msq - mean^2/D ; std = sqrt(max(var2,0)/(D-1))
                ve.scalar_tensor_tensor(out=var[:, cs], in0=m2[:, cs],
                                        scalar=float(-1.0 / D), in1=msq[:, cs],
                                        op0=OP.mult, op1=OP.add)
                ve.tensor_scalar(out=var[:, cs], in0=var[:, cs], scalar1=0.0,
                                 scalar2=None, op0=OP.max)
                act.activation(std[:, cs], var[:, cs], AF.Sqrt,
                               scale=float(1.0 / (D - 1)))
                ve.tensor_scalar(out=std[:, cs], in0=std[:, cs], scalar1=1e-6,
                                 scalar2=None, op0=OP.add)
                ve.reciprocal_approx_fast(istd[:, cs], std[:, cs])
                ve.scalar_tensor_tensor(out=negmistd[:, cs], in0=mean[:, cs],
                                        scalar=float(-1.0 / D),
                                        in1=istd[:, cs], op0=OP.mult, op1=OP.mult)
                istd_b = istd[:, cs].unsqueeze(2).broadcast_to([128, n, K])
                ve.tensor_tensor(pgi3[:, cs, :], pg3[:, cs, :], istd_b, OP.mult)
                pgi_b2 = pgi3[:, cs, :].unsqueeze(3).broadcast_to([128, n, K, 2])
                ve.tensor_copy(pgiX4[:, cs, :, :], pgi_b2)
                # pair-expanded negmistd (bf16) for the 2x add
                nm_b2 = negmistd[:, cs].unsqueeze(2).broadcast_to([128, n, 2])
                ve.tensor_copy(nmX3[:, cs, :], nm_b2)

            def final(jq):
                for q in range(4):
                    b = 4 * jq + q
                    x_ex = xep.tile([128, NPT], BF16, name="x_ex", tag="x_ex")
                    out_ex = op_.tile([128, NPT], BF16, name="out_ex", tag="out_ex")
                    # X = mask * pgi  (bf16)
                    if X_ENG[b] == 've':
                        # 4 per-k ops, all operands packed -> 2x_1p
                        xk = x_ex.rearrange("p (t k c2 w) -> p k t c2 w",
                                            t=T_PER_EX, k=K, w=2)
                        mk = masks5[:, jq, :, q, :] \
                            .rearrange("p t (c2 w) -> p t c2 w", w=2)
                        for k in range(K):
                            pk = pgiX4[:, 8 * b:8 * b + 8, k, :].unsqueeze(2) \
                                .broadcast_to([128, T_PER_EX, C // 2, 2])
                            ve.tensor_tensor(xk[:, k], mk, pk, OP.mult)
                    else:
                        x4v = x_ex.rearrange("p (t k c) -> p t k c",
                                             t=T_PER_EX, c=C)
                        mask_b = masks5[:, jq, :, q, :].unsqueeze(2) \
                            .broadcast_to([128, T_PER_EX, K, C])
                        pgi_b = pgi3[:, 8 * b:8 * b + 8, :].unsqueeze(3) \
                            .broadcast_to([128, T_PER_EX, K, C])
                        gp.tensor_tensor(x4v, mask_b, pgi_b, OP.mult)
                    # out = X + negmistd
                    if ADD_ENG[b] == 've':
                        ow = out_ex.rearrange("p (t f w) -> p t f w",
                                              t=T_PER_EX, w=2)
                        xw = x_ex.rearrange("p (t f w) -> p t f w",
                                            t=T_PER_EX, w=2)
                        nw = nmX3[:, 8 * b:8 * b + 8, :].unsqueeze(2) \
                            .broadcast_to([128, T_PER_EX, 64, 2])
                        ve.tensor_tensor(ow, xw, nw, OP.add)
                    else:
                        for t in range(T_PER_EX):
                            jg = 8 * b + t
                            act.activation(out_ex[:, D * t:D * (t + 1)],
                                           x_ex[:, D * t:D * (t + 1)],
                                           AF.Identity,
                                           bias=negmistd[:, jg:jg + 1], scale=1.0)
                    dst = out_d.rearrange("b (p t) j -> b p (t j)", p=128)[b]
                    sy.dma_start(dst, out_ex)

            preamble(0)
            preamble(1)
            stats_pair(0)
            preamble(2)
            preamble(3)
            stats_pair(1)
            for jp in range(4):
                if jp + 2 < 4:
                    preamble(2 * jp + 4)
                    preamble(2 * jp + 5)
                    stats_pair(jp + 2)
                final(2 * jp)
                final(2 * jp + 1)

    if not nc.is_finalized():
        nc.finalize()
    return nc


_NC = None


def _get_nc():
    global _NC
    if _NC is None:
        _NC = build_nc()
    return _NC


def make_in_maps(x: np.ndarray) -> list:
    """Host permute: xp[p, b, t, c] = x[b, 8p+t, c], per core."""
    x = np.ascontiguousarray(np.asarray(x, dtype=np.float32))
    B = x.shape[0]
    per = B // 8
    cf = _consts_f32()
    xp_all = x.reshape(B, 128, T_PER_EX, 4)
    return [
        {"x": np.ascontiguousarray(
            xp_all[i * per:(i + 1) * per].transpose(1, 0, 2, 3)
         ).reshape(128, per * T_PER_EX * 4),
         "cf": cf} for i in range(8)
    ]


def kernel(x: np.ndarray) -> np.ndarray:
    from concourse.bass_utils import run_bass_kernel_spmd

    n_cores = 8
    nc = _get_nc()
    in_maps = make_in_maps(x)
    res = run_bass_kernel_spmd(nc, in_maps, core_ids=list(range(n_cores)))
    return np.concatenate(
        [np.asarray(r["out"]).astype(np.float32) for r in res.results], axis=0)


if __name__ == "__main__":
    from concourse.bass_interp import CoreSim

    rng = np.random.default_rng(0)
    x = (rng.standard_normal((EX, NPT, 4)) * 2.0).astype(np.float32)
    x[..., 3] = rng.integers(0, C, size=(EX, NPT)).astype(np.float32)
    nc = build_nc()
    sim = CoreSim(nc)
    xp = np.ascontiguousarray(
        x.reshape(EX, 128, T_PER_EX, 4).transpose(1, 0, 2, 3)
    ).reshape(128, EX * T_PER_EX * 4)
    sim.tensor("x")[:] = xp
    sim.tensor("cf")[:] = _consts_f32()
    sim.simulate()
    got = np.array(sim.tensor("out")).astype(np.float32)

    xyz = x[..., :3]; clsf_ = x[..., 3]
    r = np.sqrt((xyz * xyz).sum(-1)); rinv = 1.0 / r
    RS = [0.0, 1.5, 3.0, 4.5]
    radial = np.exp(-ETA * (np.array(RS, np.float32)[None, None] - r[..., None]) ** 2)
    env = 0.5 * np.cos(np.pi * np.minimum(r, RC) / RC) + 0.5
    sh = np.stack([np.full_like(r, Y00), C1 * xyz[..., 1] * rinv,
                   C1 * xyz[..., 2] * rinv, C1 * xyz[..., 0] * rinv], -1)
    prod = sh * radial * env[..., None]
    onehot = (clsf_[..., None] == np.arange(C, dtype=np.float32)).astype(np.float32)
    pos = (prod[..., :, None] * onehot[..., None, :]).reshape(EX, NPT, D)
    norm = np.sqrt((pos * pos).sum(1, keepdims=True))
    pos = pos / np.maximum(norm, 1e-12)
    mean_ = pos.mean(-1, keepdims=True)
    std_ = pos.std(-1, ddof=1, keepdims=True)
    want = (pos - mean_) / (std_ + 1e-6)
    err = np.abs(got - want)
    print("sim absmax err:", err.max(), "ref absmax:", np.abs(want).max())
    rel = np.linalg.norm((got - want).ravel()) / np.linalg.norm(want.ravel())
    print("sim rel err:", rel)
